# revision 40
# baseline (speedup 1.0000x reference)
"""GAT (2-layer dense-graph attention over 4096 nodes) as a Trainium2
Bass/Tile SPMD kernel across 8 NeuronCores.

Structure:
- Layer 0 DST-sharded (512 destination rows/core, full 4096-source
  h'/d per core). Layer 1 SOURCE-sharded: each core's own 512 layer-0
  output rows are its layer-1 sources; partial numerators/denominators
  for ALL 4096 destinations are summed with one bf16 ReduceScatter.
- Collectives: tiny s1 AllGather (2KB/core) + final ReduceScatter. A
  dummy AllGather at kernel start absorbs the device barrier + CC
  warmup off the critical path.
- x arrives host-transposed (xT [32,4096]) so all DMAs are contiguous
  16KB-per-partition reads - no on-chip transposes of x, no strided
  gather DMA.
- E' = max(e^{0.8 s_i} e^{d_j}, e^{0.2 d_j}) (exact leakyrelu-softmax
  rescale) as ONE DVE tensor_scalar per tile, bf16.
- elu as contp = elu(x)+1 = max(x,0) + min(e^x,1); the -1 folded into
  layer-1 beta rows. All biases/BN folded host-side.
- Reciprocals via DVE reciprocal_approx_fast (~18 bits, one op):
  ScalarE uses only Exp/Copy -> a single ACT table set, zero reloads.
- d0 scores accumulate into a persistent PSUM bank; two batched
  ScalarE exps produce all 256 per-(jt,h) softmax scalars.
- Dummy matmuls keep the PE HAM clock-gate warm across the s1-gather
  and ReduceScatter waits.
"""

import numpy as np
import ml_dtypes

import concourse.bacc as bacc
import concourse.mybir as mybir
import concourse.tile as tile
from concourse import masks
from concourse.bass_utils import run_bass_kernel_spmd

F32 = mybir.dt.float32
BF16 = mybir.dt.bfloat16
AF = mybir.ActivationFunctionType
OP = mybir.AluOpType
N = 4096
NCORES = 8
RPC = N // NCORES          # rows per core = 512
NJT = N // 128             # 32 j-tiles of 128 source rows
BN_EPS = 1e-5

_CACHE = {}


def _build():
    nc = bacc.Bacc("TRN2", target_bir_lowering=False, debug=False,
                   num_devices=NCORES)

    xT_d = nc.dram_tensor("xT", [33, N], F32, kind="ExternalInput")
    xsT_d = nc.dram_tensor("xsT", [33, RPC], F32, kind="ExternalInput")
    w0all_d = nc.dram_tensor("w0all", [33, 72], F32, kind="ExternalInput")
    w0s_d = nc.dram_tensor("w0s", [33, 8], F32, kind="ExternalInput")
    w1ext_d = nc.dram_tensor("w1ext", [65, 65], F32, kind="ExternalInput")
    sela_d = nc.dram_tensor("sela", [8, 8 * 128], BF16, kind="ExternalInput")
    s2sel_d = nc.dram_tensor("s2sel", [2, 16], F32, kind="ExternalInput")
    out_d = nc.dram_tensor("out", [RPC, 32], F32, kind="ExternalOutput")

    with tile.TileContext(nc) as tc:
        with (
            tc.tile_pool(name="const", bufs=1) as const,
            tc.tile_pool(name="per", bufs=1) as per,
            tc.tile_pool(name="psper", bufs=1, space="PSUM") as psper,
            tc.tile_pool(name="dram", bufs=1, space="DRAM") as dram,
        ):
            # ---------- dram intermediates ----------
            dum_i = dram.tile([1, 8], F32, name="dum_i", tag="dum_i")
            dum_o = dram.tile([8, 8], F32, name="dum_o", tag="dum_o")
            s1d = dram.tile([1, RPC], F32, name="s1d", tag="s1d")
            s1g = dram.tile([NCORES, RPC], F32, name="s1g", tag="s1g")
            rsin = dram.tile([NCORES, 33, RPC], BF16, name="rsin", tag="rsin")
            rsout = dram.tile([33, RPC], BF16, name="rsout", tag="rsout")

            # dummy collective first: absorbs device barrier + CC warmup
            nc.gpsimd.collective_compute(
                "AllGather", OP.bypass,
                replica_groups=[list(range(NCORES))],
                ins=[dum_i.opt()], outs=[dum_o.opt()])

            # ---------- consts ----------
            ident = const.tile([128, 128], F32)
            masks.make_identity(nc, ident[:])
            ones512 = const.tile([1, RPC], F32)
            nc.vector.memset(ones512[:], 1.0)
            ones32 = const.tile([1, 32], F32)
            nc.vector.memset(ones32[:], 1.0)
            sela = const.tile([8, 8 * 128], BF16)
            nc.sync.dma_start(sela[:], sela_d[:])
            s2sel = const.tile([2, 16], F32)
            nc.sync.dma_start(s2sel[:], s2sel_d[:])
            w0all = const.tile([33, 72], F32)
            nc.sync.dma_start(w0all[:], w0all_d[:])
            w0s = const.tile([33, 8], F32)
            nc.sync.dma_start(w0s[:], w0s_d[:])
            w1c = [const.tile([16, 65], F32, name=f"w1c{c}", tag=f"w1c{c}")
                   for c in range(4)]
            for c in range(4):
                nc.sync.dma_start(w1c[c][:], w1ext_d[16 * c:16 * c + 16, :])
            w1last = const.tile([1, 65], F32)
            nc.sync.dma_start(w1last[:], w1ext_d[64:65, :])

            # ---------- persistent sbuf ----------
            xT = per.tile([33, N], F32)
            xsT = per.tile([33, RPC], F32)
            # stationary holds h'*e^{d} (cols 0:8) and e^{d} (col 32) so
            # the per-tile DVE op is a SINGLE-scalar max:
            #   E'' = max(e^{0.8 s_i}, e^{-0.8 d_j});  E = e^{d_j} * E''
            hpa0 = per.tile([128, NJT, 8, 34], BF16)
            g0 = per.tile([128, NJT * 8], F32)         # e^{-0.8 d0}
            b0rep = per.tile([128, NJT, 8, 8], F32)    # e^{d0} rep x8
            hp0f = per.tile([128, NJT, 64], F32)       # h'0 staging
            atile = per.tile([128, 8, RPC], BF16)      # e^{0.8 s0} bcast
            nums = per.tile([16, 4, RPC], F32)
            dens = per.tile([2, 4, RPC], F32)
            contp = per.tile([16, 4, RPC], F32)        # elu(out0)+1 chunks
            hp1s = per.tile([33, RPC], F32)
            hpa1 = per.tile([128, 4, 34], BF16)
            g1e = per.tile([128, 4], F32)              # e^{-0.8 d1}
            b1e = per.tile([128, 4], F32)              # e^{d1}
            s1s = per.tile([1, RPC], F32)
            s1raw = per.tile([8, RPC], F32)
            a1g = per.tile([8, RPC], BF16)
            a1t = per.tile([128, 8, RPC], BF16)
            num32 = per.tile([32, RPC], F32)
            outv = per.tile([32, RPC], F32)
            rso = per.tile([33, RPC], BF16)
            rcp1 = per.tile([1, RPC], F32)

            ps1 = psper.tile([65, RPC], F32)
            hT72 = per.tile([72, N], F32)              # h'0/d0 row-major
            d0raw = per.tile([128, NJT * 8], F32)      # d0 scores, j-major

            # ---------------- Phase A: warmup + prep ----------------
            with (
                tc.tile_pool(name="ld", bufs=2) as ld,
                tc.tile_pool(name="mm72", bufs=2, space="PSUM") as mm72,
                tc.tile_pool(name="trp", bufs=2, space="PSUM") as trp,
                tc.tile_pool(name="ps0p", bufs=1, space="PSUM") as ps0p,
                tc.tile_pool(name="pab", bufs=2, space="PSUM") as pab,
            ):
                wsrc = ld.tile([128, 512], BF16, tag="wsrc")
                nc.vector.memset(wsrc[:], 0.5)
                wlhs = ld.tile([128, 128], BF16, tag="wlhs")
                nc.vector.memset(wlhs[:], 0.25)
                wps = pab.tile([128, RPC], F32, tag="pa")
                for r in range(14):
                    nc.tensor.matmul(wps[:], wlhs[:], wsrc[:],
                                     start=(r == 0), stop=(r == 13))

                # host-transposed inputs (ones row baked in on host):
                # contiguous big-granule DMAs, zero on-chip fixup
                nc.sync.dma_start(xT[:], xT_d[:])
                nc.sync.dma_start(xsT[:], xsT_d[:])

                # s0 for own 512 dst rows; atile = e^{0.8 s0} bcast
                ps0 = ps0p.tile([8, RPC], F32, tag="ps0")
                nc.tensor.matmul(ps0[:], w0s[:], xsT[:])
                a0row = ld.tile([8, RPC], BF16, tag="a0row")
                nc.scalar.activation(a0row[:], ps0[:], AF.Exp, scale=0.8)
                for h in range(8):
                    pa = pab.tile([128, RPC], F32, tag="pa")
                    nc.tensor.matmul(pa[:], sela[:, h * 128:(h + 1) * 128],
                                     a0row[:])
                    nc.scalar.copy(atile[:, h, :], pa[:])

                # h'0/d0 for all 4096 sources: ONE 72-col stationary
                # (w0all), xT streamed through in 8 chunks -> row-major
                # [72, 4096]; PE transposes bring it back j-on-partitions
                nc.vector.memset(hpa0[:], 0.0)
                nc.vector.memset(hpa0[:, :, :, 32:33], 1.0)
                for cc in range(8):
                    p72 = mm72.tile([72, 512], F32, tag="p72")
                    nc.tensor.matmul(p72[:], w0all[:],
                                     xT[:, cc * 512:(cc + 1) * 512])
                    nc.scalar.copy(hT72[:, cc * 512:(cc + 1) * 512], p72[:])
                for jt in range(NJT):
                    ptr = trp.tile([128, 72], F32, tag="ptr")
                    nc.tensor.matmul(ptr[:],
                                     hT72[:, jt * 128:(jt + 1) * 128],
                                     ident[0:72, 0:72], is_transpose=True)
                    nc.vector.tensor_copy(hp0f[:, jt, :], ptr[:, 0:64])
                    nc.vector.tensor_copy(d0raw[:, jt * 8:(jt + 1) * 8],
                                          ptr[:, 64:72])
                    if jt % 8 == 7:
                        g = jt // 8
                        gs = slice(g * 64, (g + 1) * 64)
                        gb = slice(g * 8, (g + 1) * 8)
                        draw = d0raw[:, gs].rearrange(
                            "p (a b) -> p a b", b=8)
                        nc.scalar.activation(g0[:, gs], d0raw[:, gs],
                                             AF.Exp, scale=-0.8)
                        nc.scalar.activation(
                            b0rep[:, gb, :, 0], draw, AF.Exp)
                        for o in range(1, 8):
                            nc.vector.tensor_copy(
                                b0rep[:, gb, :, o], b0rep[:, gb, :, 0])
                        nc.vector.tensor_copy(
                            hpa0[:, gb, :, 32], b0rep[:, gb, :, 0])
                        nc.vector.tensor_tensor(
                            hpa0[:, gb, :, 0:8],
                            hp0f[:, gb, :].rearrange(
                                "p a (h o) -> p a h o", h=8),
                            b0rep[:, gb, :, :], op=OP.mult)

            # ---------------- Phase B: layer-0 attention ----------------
            with (
                tc.tile_pool(name="epool", bufs=8) as epool,
                tc.tile_pool(name="agg", bufs=3, space="PSUM") as agg,
                tc.tile_pool(name="rb", bufs=1, space="PSUM") as rb,
                tc.tile_pool(name="tmp", bufs=2) as tmp,
            ):
                for h in range(8):
                    ch, hh = h // 2, h % 2
                    pg = agg.tile([33, RPC], F32)
                    for jt in range(NJT):
                        e = epool.tile([128, RPC], BF16, tag="e")
                        nc.vector.tensor_scalar(
                            e[:], atile[:, h, :],
                            g0[:, jt * 8 + h:jt * 8 + h + 1], None,
                            op0=OP.max)
                        nc.tensor.matmul(pg[:], hpa0[:, jt, h, 0:33], e[:],
                                         start=(jt == 0), stop=(jt == NJT - 1))
                    stg = tmp.tile([8, RPC], F32, tag="stg")
                    nc.scalar.copy(stg[:], pg[0:8, :])
                    std = tmp.tile([1, RPC], F32, tag="std")
                    nc.scalar.copy(std[:], pg[32:33, :])
                    nc.sync.dma_start(nums[hh * 8:(hh + 1) * 8, ch, :], stg[:])
                    nc.sync.dma_start(dens[hh:hh + 1, ch, :], std[:])

                    if hh == 1:
                        # chunk complete: normalize + (elu+1) -> contp
                        rcp = tmp.tile([2, RPC], F32, tag="rcp")
                        nc.vector.reciprocal_approx_fast(rcp[:], dens[:, ch, :])
                        prbc = rb.tile([16, RPC], F32)
                        nc.tensor.matmul(prbc[:], s2sel[:], rcp[:])
                        nrm = tmp.tile([16, RPC], F32, tag="nrm")
                        nc.vector.tensor_tensor(nrm[:], nums[:, ch, :],
                                                prbc[:], op=OP.mult)
                        texp = tmp.tile([16, RPC], F32, tag="texp")
                        nc.scalar.activation(texp[:], nrm[:], AF.Exp)
                        t1 = tmp.tile([16, RPC], F32, tag="t1")
                        nc.vector.tensor_scalar_min(t1[:], texp[:], 1.0)
                        nc.vector.scalar_tensor_tensor(
                            contp[:, ch, :], nrm[:], 0.0, t1[:],
                            op0=OP.max, op1=OP.add)
                        # accumulate s1 / h'1 / d1 into ps1
                        nc.tensor.matmul(ps1[:], w1c[ch][:], contp[:, ch, :],
                                         start=(ch == 0), stop=(ch == 3))
                        if ch == 0:
                            nc.tensor.matmul(ps1[:], w1last[:], ones512[:],
                                             start=False, stop=False)

            # ---------------- Phase C: inter-layer + layer 1 ----------------
            with (
                tc.tile_pool(name="ld2", bufs=2) as ld2,
                tc.tile_pool(name="tp2", bufs=2, space="PSUM") as tp2,
                tc.tile_pool(name="pa1p", bufs=2, space="PSUM") as pa1p,
                tc.tile_pool(name="agg1", bufs=2, space="PSUM") as agg1,
                tc.tile_pool(name="e1pool", bufs=4) as e1pool,
                tc.tile_pool(name="rssp", bufs=2) as rssp,
                tc.tile_pool(name="otp", bufs=2) as otp,
            ):
                # bridge the terminal-chunk serial chain so HAM stays hot
                wsrcb = ld2.tile([128, 512], BF16, tag="wsrcb")
                nc.vector.memset(wsrcb[:], 0.5)
                wlhsb = ld2.tile([128, 128], BF16, tag="wlhsb")
                nc.vector.memset(wlhsb[:], 0.25)
                wpsb = pa1p.tile([128, RPC], F32, tag="pa1")
                for r in range(16):
                    nc.tensor.matmul(wpsb[:], wlhsb[:], wsrcb[:],
                                     start=(r == 0), stop=(r == 15))

                # s1 out the door first: evac -> DRAM -> AllGather
                nc.scalar.copy(s1s[:], ps1[64:65, :])
                nc.sync.dma_start(s1d[:], s1s[:])
                nc.gpsimd.collective_compute(
                    "AllGather", OP.bypass,
                    replica_groups=[list(range(NCORES))],
                    ins=[s1d.opt()], outs=[s1g.opt()])
                nc.sync.dma_start(s1raw[:], s1g[:])

                # local h'1 -> transposed bf16 stationary + d1 exps
                nc.scalar.copy(hp1s[0:32, :], ps1[0:32, :])
                nc.scalar.copy(hp1s[32:33, :], ps1[32:33, :])
                for q in range(4):
                    ptq = tp2.tile([128, 33], F32, tag="ptq")
                    nc.tensor.matmul(ptq[:],
                                     hp1s[:, q * 128:(q + 1) * 128],
                                     ident[0:33, 0:33], is_transpose=True)
                    nc.scalar.activation(g1e[:, q:q + 1], ptq[:, 32:33],
                                         AF.Exp, scale=-0.8)
                    nc.scalar.activation(b1e[:, q:q + 1], ptq[:, 32:33],
                                         AF.Exp)
                    nc.vector.tensor_scalar(
                        hpa1[:, q, 0:32], ptq[:, 0:32],
                        b1e[:, q:q + 1], None, op0=OP.mult)
                    nc.vector.tensor_copy(hpa1[:, q, 32:33],
                                          b1e[:, q:q + 1])
                nc.vector.memset(hpa1[:, :, 33:34], 0.0)

                # keep PE hot while the s1 AllGather is in flight
                wps2 = tp2.tile([128, 33], F32, tag="ptq")
                for r in range(80):
                    nc.tensor.matmul(wps2[:], wlhsb[:], wsrcb[:, 0:33],
                                     start=(r == 0), stop=(r == 79))

                # gathered s1 -> a1 = e^{0.8 s1} (bf16), bcast per dst chunk
                nc.scalar.activation(a1g[:], s1raw[:], AF.Exp, scale=0.8)

                for p in range(4):
                    for k in range(2):
                        ic = 2 * p + k
                        pa1 = pa1p.tile([128, RPC], F32, tag="pa1")
                        nc.tensor.matmul(pa1[:],
                                         sela[:, ic * 128:(ic + 1) * 128],
                                         a1g[:])
                        nc.vector.tensor_copy(a1t[:, ic, :], pa1[:])
                    pgs = [agg1.tile([33, RPC], F32, name=f"pg1_{p}_{k}",
                                     tag="pg1") for k in range(2)]
                    for jt in range(4):
                        e1 = e1pool.tile([128, 2 * RPC], BF16, tag="e1")
                        nc.vector.tensor_scalar(
                            e1[:].rearrange("p (a b) -> p a b", b=RPC),
                            a1t[:, 2 * p:2 * p + 2, :],
                            g1e[:, jt:jt + 1], None, op0=OP.max)
                        for k in range(2):
                            nc.tensor.matmul(
                                pgs[k][:], hpa1[:, jt, 0:33],
                                e1[:, k * RPC:(k + 1) * RPC],
                                start=(jt == 0), stop=(jt == 3))
                    for k in range(2):
                        rss = rssp.tile([33, RPC], BF16, tag="rss")
                        nc.vector.tensor_copy(rss[0:32, :], pgs[k][0:32, :])
                        nc.vector.tensor_copy(rss[32:33, :],
                                              pgs[k][32:33, :])
                        nc.sync.dma_start(rsin[2 * p + k, :, :], rss[:])

                # keep PE hot across the ReduceScatter
                wps3 = pa1p.tile([128, RPC], F32, tag="pa1")
                for r in range(40):
                    nc.tensor.matmul(wps3[:], wlhsb[:], wsrcb[:],
                                     start=(r == 0), stop=(r == 39))

                nc.gpsimd.collective_compute(
                    "ReduceScatter", OP.add,
                    replica_groups=[list(range(NCORES))],
                    ins=[rsin.opt()], outs=[rsout.opt()])
                nc.sync.dma_start(rso[:], rsout[:])

                # normalize + write out
                nc.scalar.copy(num32[:], rso[0:32, :])
                nc.scalar.copy(rcp1[:], rso[32:33, :])
                nc.vector.reciprocal_approx_fast(rcp1[:], rcp1[:])
                prb1 = agg1.tile([33, RPC], F32, tag="pg1")
                nc.tensor.matmul(prb1[0:32, :], ones32[:], rcp1[:])
                nc.vector.tensor_tensor(outv[:], num32[:], prb1[0:32, :],
                                        op=OP.mult)
                for q in range(4):
                    pt2 = tp2.tile([128, 33], F32, tag="ptq")
                    nc.tensor.matmul(pt2[:, 0:32],
                                     outv[:, q * 128:(q + 1) * 128],
                                     ident[0:32, 0:32], is_transpose=True)
                    ob = otp.tile([128, 32], F32, tag="ob")
                    nc.vector.tensor_copy(ob[:], pt2[:, 0:32])
                    nc.sync.dma_start(out_d[q * 128:(q + 1) * 128, :], ob[:])

    nc.compile()
    return nc


def _fold(inputs):
    """Host-side BN/bias/attention-projection folding (numpy, f64)."""
    f64 = np.float64
    x = np.asarray(inputs["x"], np.float32)
    xT = np.ascontiguousarray(x.T)              # [32, 4096]
    w0 = np.asarray(inputs["w0"], f64)          # [8, 32, 8]
    w1 = np.asarray(inputs["w1"], f64)          # [1, 64, 32]
    a_src0 = np.asarray(inputs["a_src0"], f64)[..., 0]   # [8, 8]
    a_dst0 = np.asarray(inputs["a_dst0"], f64)[..., 0]   # [8, 8]
    a_src1 = np.asarray(inputs["a_src1"], f64)[0, :, 0]  # [32]
    a_dst1 = np.asarray(inputs["a_dst1"], f64)[0, :, 0]  # [32]
    b0 = np.asarray(inputs["b0"], f64)          # [8]
    b1 = np.asarray(inputs["b1"], f64)          # [32]

    al0 = np.asarray(inputs["bn0_gamma"], f64) / np.sqrt(
        np.asarray(inputs["bn0_var"], f64) + BN_EPS)
    sh0 = np.asarray(inputs["bn0_beta"], f64) - \
        np.asarray(inputs["bn0_mean"], f64) * al0
    al1 = np.asarray(inputs["bn1_gamma"], f64) / np.sqrt(
        np.asarray(inputs["bn1_var"], f64) + BN_EPS)
    sh1 = np.asarray(inputs["bn1_beta"], f64) - \
        np.asarray(inputs["bn1_mean"], f64) * al1

    w0flat = (al0[None, :, None] * w0).transpose(1, 0, 2).reshape(32, 64)
    beta0h = np.einsum("i,hio->ho", sh0, w0)     # [8, 8]
    beta0 = (beta0h + b0[None, :]).reshape(64)
    as0 = al0[:, None] * np.einsum("hio,ho->ih", w0, a_src0)   # [32, 8]
    sb0 = np.einsum("ho,ho->h", beta0h, a_src0)
    ad0 = al0[:, None] * np.einsum("hio,ho->ih", w0, a_dst0)
    db0 = np.einsum("ho,ho->h", beta0h, a_dst0)

    w0all = np.zeros((33, 72), f64)
    w0all[0:32, 0:64] = w0flat
    w0all[32, 0:64] = beta0
    w0all[0:32, 64:72] = ad0
    w0all[32, 64:72] = db0
    w0s = np.zeros((33, 8), f64)
    w0s[0:32, :] = as0
    w0s[32, :] = sb0

    # layer 1 folds; input arrives as contp = elu(out0)+1
    w1m = w1[0]                                   # [64, 32]
    w1flat = al1[:, None] * w1m
    beta1 = sh1 @ w1m + b1
    as1 = al1 * (w1m @ a_src1)
    sb1 = (sh1 @ w1m) @ a_src1
    ad1 = al1 * (w1m @ a_dst1)
    db1 = (sh1 @ w1m) @ a_dst1

    w1ext = np.zeros((65, 65), f64)
    w1ext[0:64, 0:32] = w1flat
    w1ext[64, 0:32] = beta1 - w1flat.sum(axis=0)
    w1ext[0:64, 32] = ad1
    w1ext[64, 32] = db1 - ad1.sum()
    w1ext[0:64, 64] = as1
    w1ext[64, 64] = sb1 - as1.sum()

    sela = np.zeros((8, 8, 128), ml_dtypes.bfloat16)
    for h in range(8):
        sela[h, h, :] = 1.0
    s2sel = np.zeros((2, 16), np.float32)
    for p in range(2):
        s2sel[p, p * 8:(p + 1) * 8] = 1.0

    xTo = np.ones((33, N), np.float32)
    xTo[0:32, :] = xT
    return {
        "xT": xTo,
        "w0all": w0all.astype(np.float32),
        "w0s": w0s.astype(np.float32),
        "w1ext": w1ext.astype(np.float32),
        "sela": sela.reshape(8, 8 * 128),
        "s2sel": s2sel,
    }


def kernel(**inputs) -> np.ndarray:
    if "nc" not in _CACHE:
        _CACHE["nc"] = _build()
    nc = _CACHE["nc"]

    shared = _fold(inputs)
    xTo = shared["xT"]
    in_maps = []
    for c in range(NCORES):
        m = dict(shared)
        m["xsT"] = np.ascontiguousarray(xTo[:, c * RPC:(c + 1) * RPC])
        in_maps.append(m)

    res = run_bass_kernel_spmd(nc, in_maps, list(range(NCORES)))
    out = np.concatenate([res.results[c]["out"] for c in range(NCORES)],
                         axis=0)
    return out.astype(np.float32)


# revision 41
# speedup vs baseline: 1.1174x; 1.1174x over previous
"""GAT (2-layer dense-graph attention over 4096 nodes) as a Trainium2
Bass/Tile SPMD kernel across 8 NeuronCores.

Structure:
- Layer 0 DST-sharded (512 destination rows/core, full 4096-source
  h'/d per core). Layer 1 SOURCE-sharded: each core's own 512 layer-0
  output rows are its layer-1 sources; partial numerators/denominators
  for ALL 4096 destinations are summed with one bf16 ReduceScatter.
- Collectives: tiny s1 AllGather (2KB/core) + final ReduceScatter. A
  dummy AllGather at kernel start absorbs the device barrier + CC
  warmup off the critical path.
- x arrives host-transposed (xT [32,4096]) so all DMAs are contiguous
  16KB-per-partition reads - no on-chip transposes of x, no strided
  gather DMA.
- E' = max(e^{0.8 s_i} e^{d_j}, e^{0.2 d_j}) (exact leakyrelu-softmax
  rescale) as ONE DVE tensor_scalar per tile, bf16.
- elu as contp = elu(x)+1 = max(x,0) + min(e^x,1); the -1 folded into
  layer-1 beta rows. All biases/BN folded host-side.
- Reciprocals via DVE reciprocal_approx_fast (~18 bits, one op):
  ScalarE uses only Exp/Copy -> a single ACT table set, zero reloads.
- d0 scores accumulate into a persistent PSUM bank; two batched
  ScalarE exps produce all 256 per-(jt,h) softmax scalars.
- Dummy matmuls keep the PE HAM clock-gate warm across the s1-gather
  and ReduceScatter waits.
"""

import numpy as np
import ml_dtypes

import concourse.bacc as bacc
import concourse.mybir as mybir
import concourse.tile as tile
from concourse import masks
from concourse.bass_utils import run_bass_kernel_spmd

F32 = mybir.dt.float32
BF16 = mybir.dt.bfloat16
AF = mybir.ActivationFunctionType
OP = mybir.AluOpType
N = 4096
NCORES = 8
RPC = N // NCORES          # rows per core = 512
NJT = N // 128             # 32 j-tiles of 128 source rows
BN_EPS = 1e-5

_CACHE = {}


def _build():
    nc = bacc.Bacc("TRN2", target_bir_lowering=False, debug=False,
                   num_devices=NCORES)

    xT_d = nc.dram_tensor("xT", [33, N], F32, kind="ExternalInput")
    xsT_d = nc.dram_tensor("xsT", [33, RPC], F32, kind="ExternalInput")
    w0all_d = nc.dram_tensor("w0all", [33, 72], F32, kind="ExternalInput")
    w0s_d = nc.dram_tensor("w0s", [33, 8], F32, kind="ExternalInput")
    w1ext_d = nc.dram_tensor("w1ext", [65, 65], F32, kind="ExternalInput")
    sela_d = nc.dram_tensor("sela", [8, 8 * 128], BF16, kind="ExternalInput")
    s2sel_d = nc.dram_tensor("s2sel", [2, 16], F32, kind="ExternalInput")
    out_d = nc.dram_tensor("out", [RPC, 32], F32, kind="ExternalOutput")

    with tile.TileContext(nc) as tc:
        with (
            tc.tile_pool(name="const", bufs=1) as const,
            tc.tile_pool(name="per", bufs=1) as per,
            tc.tile_pool(name="psper", bufs=1, space="PSUM") as psper,
            tc.tile_pool(name="dram", bufs=1, space="DRAM") as dram,
        ):
            # ---------- dram intermediates ----------
            dum_i = dram.tile([1, 8], F32, name="dum_i", tag="dum_i")
            dum_o = dram.tile([8, 8], F32, name="dum_o", tag="dum_o")
            s1d = dram.tile([1, RPC], F32, name="s1d", tag="s1d")
            s1g = dram.tile([NCORES, RPC], F32, name="s1g", tag="s1g")
            rsin = dram.tile([NCORES, 33, RPC], BF16, name="rsin", tag="rsin")
            rsout = dram.tile([33, RPC], BF16, name="rsout", tag="rsout")

            # dummy collective first: absorbs device barrier + CC warmup
            nc.gpsimd.collective_compute(
                "AllGather", OP.bypass,
                replica_groups=[list(range(NCORES))],
                ins=[dum_i.opt()], outs=[dum_o.opt()])

            # ---------- consts ----------
            ident = const.tile([128, 128], F32)
            masks.make_identity(nc, ident[:])
            ones512 = const.tile([1, RPC], F32)
            nc.vector.memset(ones512[:], 1.0)
            ones32 = const.tile([1, 32], F32)
            nc.vector.memset(ones32[:], 1.0)
            sela = const.tile([8, 8 * 128], BF16)
            nc.sync.dma_start(sela[:], sela_d[:])
            s2sel = const.tile([2, 16], F32)
            nc.sync.dma_start(s2sel[:], s2sel_d[:])
            w0all = const.tile([33, 72], F32)
            nc.sync.dma_start(w0all[:], w0all_d[:])
            w0s = const.tile([33, 8], F32)
            nc.sync.dma_start(w0s[:], w0s_d[:])
            w1c = [const.tile([16, 65], F32, name=f"w1c{c}", tag=f"w1c{c}")
                   for c in range(4)]
            for c in range(4):
                nc.sync.dma_start(w1c[c][:], w1ext_d[16 * c:16 * c + 16, :])
            w1last = const.tile([1, 65], F32)
            nc.sync.dma_start(w1last[:], w1ext_d[64:65, :])

            # ---------- persistent sbuf ----------
            xT = per.tile([33, N], F32)
            xsT = per.tile([33, RPC], F32)
            # stationary holds h'*e^{d} (cols 0:8) and e^{d} (col 32) so
            # the per-tile DVE op is a SINGLE-scalar max:
            #   E'' = max(e^{0.8 s_i}, e^{-0.8 d_j});  E = e^{d_j} * E''
            hpa0 = per.tile([128, NJT, 8, 34], BF16)
            g0 = per.tile([128, NJT * 8], F32)         # e^{-0.8 d0}
            b0rep = per.tile([128, NJT, 8, 8], F32)    # e^{d0} rep x8
            hp0f = per.tile([128, NJT, 64], F32)       # h'0 staging
            atile = per.tile([128, 8, RPC], BF16)      # e^{0.8 s0} bcast
            nums = per.tile([16, 4, RPC], F32)
            dens = per.tile([2, 4, RPC], F32)
            contp = per.tile([16, 4, RPC], F32)        # elu(out0)+1 chunks
            hp1s = per.tile([33, RPC], F32)
            hpa1 = per.tile([128, 4, 34], BF16)
            g1e = per.tile([128, 4], F32)              # e^{-0.8 d1}
            b1e = per.tile([128, 4], F32)              # e^{d1}
            s1s = per.tile([1, RPC], F32)
            s1raw = per.tile([8, RPC], F32)
            a1g = per.tile([8, RPC], BF16)
            a1t = per.tile([128, 8, RPC], BF16)
            num32 = per.tile([32, RPC], F32)
            outv = per.tile([32, RPC], F32)
            rso = per.tile([33, RPC], BF16)
            rcp1 = per.tile([1, RPC], F32)

            ps1 = psper.tile([65, RPC], F32)
            hT72 = per.tile([72, N], F32)              # h'0/d0 row-major
            d0raw = per.tile([128, NJT * 8], F32)      # d0 scores, j-major

            # ---------------- Phase A: warmup + prep ----------------
            with (
                tc.tile_pool(name="ld", bufs=2) as ld,
                tc.tile_pool(name="mm72", bufs=2, space="PSUM") as mm72,
                tc.tile_pool(name="trp", bufs=2, space="PSUM") as trp,
                tc.tile_pool(name="ps0p", bufs=1, space="PSUM") as ps0p,
                tc.tile_pool(name="pab", bufs=2, space="PSUM") as pab,
            ):
                wsrc = ld.tile([128, 512], BF16, tag="wsrc")
                nc.vector.memset(wsrc[:], 0.5)
                wlhs = ld.tile([128, 128], BF16, tag="wlhs")
                nc.vector.memset(wlhs[:], 0.25)
                wps = pab.tile([128, RPC], F32, tag="pa")
                for r in range(14):
                    nc.tensor.matmul(wps[:], wlhs[:], wsrc[:],
                                     start=(r == 0), stop=(r == 13))

                # host-transposed inputs (ones row baked in on host):
                # contiguous big-granule DMAs, zero on-chip fixup
                nc.sync.dma_start(xT[:], xT_d[:])
                nc.sync.dma_start(xsT[:], xsT_d[:])

                # s0 for own 512 dst rows; atile = e^{0.8 s0} bcast
                ps0 = ps0p.tile([8, RPC], F32, tag="ps0")
                nc.tensor.matmul(ps0[:], w0s[:], xsT[:])
                a0row = ld.tile([8, RPC], BF16, tag="a0row")
                nc.scalar.activation(a0row[:], ps0[:], AF.Exp, scale=0.8)
                for h in range(8):
                    pa = pab.tile([128, RPC], F32, tag="pa")
                    nc.tensor.matmul(pa[:], sela[:, h * 128:(h + 1) * 128],
                                     a0row[:])
                    nc.scalar.copy(atile[:, h, :], pa[:])

                # h'0/d0 for all 4096 sources: ONE 72-col stationary
                # (w0all), xT streamed through in 8 chunks -> row-major
                # [72, 4096]; PE transposes bring it back j-on-partitions
                nc.vector.memset(hpa0[:], 0.0)
                nc.vector.memset(hpa0[:, :, :, 32:33], 1.0)
                for cc in range(8):
                    p72 = mm72.tile([72, 512], F32, tag="p72")
                    nc.tensor.matmul(p72[:], w0all[:],
                                     xT[:, cc * 512:(cc + 1) * 512])
                    nc.scalar.copy(hT72[:, cc * 512:(cc + 1) * 512], p72[:])
                for jt in range(NJT):
                    ptr = trp.tile([128, 72], F32, tag="ptr")
                    nc.tensor.matmul(ptr[:],
                                     hT72[:, jt * 128:(jt + 1) * 128],
                                     ident[0:72, 0:72], is_transpose=True)
                    nc.vector.tensor_copy(hp0f[:, jt, :], ptr[:, 0:64])
                    nc.vector.tensor_copy(d0raw[:, jt * 8:(jt + 1) * 8],
                                          ptr[:, 64:72])
                    if jt % 8 == 7:
                        g = jt // 8
                        gs = slice(g * 64, (g + 1) * 64)
                        gb = slice(g * 8, (g + 1) * 8)
                        draw = d0raw[:, gs].rearrange(
                            "p (a b) -> p a b", b=8)
                        nc.scalar.activation(g0[:, gs], d0raw[:, gs],
                                             AF.Exp, scale=-0.8)
                        for o in range(8):
                            nc.scalar.activation(
                                b0rep[:, gb, :, o], draw, AF.Exp)
                        nc.scalar.activation(
                            hpa0[:, gb, :, 32], draw, AF.Exp)
                        nc.vector.tensor_tensor(
                            hpa0[:, gb, :, 0:8],
                            hp0f[:, gb, :].rearrange(
                                "p a (h o) -> p a h o", h=8),
                            b0rep[:, gb, :, :], op=OP.mult)

            # ---------------- Phase B: layer-0 attention ----------------
            with (
                tc.tile_pool(name="epool", bufs=8) as epool,
                tc.tile_pool(name="agg", bufs=3, space="PSUM") as agg,
                tc.tile_pool(name="rb", bufs=1, space="PSUM") as rb,
                tc.tile_pool(name="tmp", bufs=2) as tmp,
            ):
                for h in range(8):
                    ch, hh = h // 2, h % 2
                    pg = agg.tile([33, RPC], F32)
                    for jt in range(NJT):
                        e = epool.tile([128, RPC], BF16, tag="e")
                        nc.vector.tensor_scalar(
                            e[:], atile[:, h, :],
                            g0[:, jt * 8 + h:jt * 8 + h + 1], None,
                            op0=OP.max)
                        nc.tensor.matmul(pg[:], hpa0[:, jt, h, 0:33], e[:],
                                         start=(jt == 0), stop=(jt == NJT - 1))
                    stg = tmp.tile([8, RPC], F32, tag="stg")
                    nc.scalar.copy(stg[:], pg[0:8, :])
                    std = tmp.tile([1, RPC], F32, tag="std")
                    nc.scalar.copy(std[:], pg[32:33, :])
                    nc.sync.dma_start(nums[hh * 8:(hh + 1) * 8, ch, :], stg[:])
                    nc.sync.dma_start(dens[hh:hh + 1, ch, :], std[:])

                    if hh == 1:
                        # chunk complete: normalize + (elu+1) -> contp
                        rcp = tmp.tile([2, RPC], F32, tag="rcp")
                        nc.vector.reciprocal_approx_fast(rcp[:], dens[:, ch, :])
                        prbc = rb.tile([16, RPC], F32)
                        nc.tensor.matmul(prbc[:], s2sel[:], rcp[:])
                        nrm = tmp.tile([16, RPC], F32, tag="nrm")
                        nc.vector.tensor_tensor(nrm[:], nums[:, ch, :],
                                                prbc[:], op=OP.mult)
                        texp = tmp.tile([16, RPC], F32, tag="texp")
                        nc.scalar.activation(texp[:], nrm[:], AF.Exp)
                        t1 = tmp.tile([16, RPC], F32, tag="t1")
                        nc.vector.tensor_scalar_min(t1[:], texp[:], 1.0)
                        nc.vector.scalar_tensor_tensor(
                            contp[:, ch, :], nrm[:], 0.0, t1[:],
                            op0=OP.max, op1=OP.add)
                        # accumulate s1 / h'1 / d1 into ps1
                        nc.tensor.matmul(ps1[:], w1c[ch][:], contp[:, ch, :],
                                         start=(ch == 0), stop=(ch == 3))
                        if ch == 0:
                            nc.tensor.matmul(ps1[:], w1last[:], ones512[:],
                                             start=False, stop=False)

            # ---------------- Phase C: inter-layer + layer 1 ----------------
            with (
                tc.tile_pool(name="ld2", bufs=2) as ld2,
                tc.tile_pool(name="tp2", bufs=2, space="PSUM") as tp2,
                tc.tile_pool(name="pa1p", bufs=2, space="PSUM") as pa1p,
                tc.tile_pool(name="agg1", bufs=2, space="PSUM") as agg1,
                tc.tile_pool(name="e1pool", bufs=4) as e1pool,
                tc.tile_pool(name="rssp", bufs=2) as rssp,
                tc.tile_pool(name="otp", bufs=2) as otp,
            ):
                # bridge the terminal-chunk serial chain so HAM stays hot
                wsrcb = ld2.tile([128, 512], BF16, tag="wsrcb")
                nc.vector.memset(wsrcb[:], 0.5)
                wlhsb = ld2.tile([128, 128], BF16, tag="wlhsb")
                nc.vector.memset(wlhsb[:], 0.25)
                wpsb = pa1p.tile([128, RPC], F32, tag="pa1")
                for r in range(16):
                    nc.tensor.matmul(wpsb[:], wlhsb[:], wsrcb[:],
                                     start=(r == 0), stop=(r == 15))

                # s1 out the door first: evac -> DRAM -> AllGather
                nc.scalar.copy(s1s[:], ps1[64:65, :])
                nc.sync.dma_start(s1d[:], s1s[:])
                nc.gpsimd.collective_compute(
                    "AllGather", OP.bypass,
                    replica_groups=[list(range(NCORES))],
                    ins=[s1d.opt()], outs=[s1g.opt()])
                nc.sync.dma_start(s1raw[:], s1g[:])

                # local h'1 -> transposed bf16 stationary + d1 exps
                nc.scalar.copy(hp1s[0:32, :], ps1[0:32, :])
                nc.scalar.copy(hp1s[32:33, :], ps1[32:33, :])
                for q in range(4):
                    ptq = tp2.tile([128, 33], F32, tag="ptq")
                    nc.tensor.matmul(ptq[:],
                                     hp1s[:, q * 128:(q + 1) * 128],
                                     ident[0:33, 0:33], is_transpose=True)
                    nc.scalar.activation(g1e[:, q:q + 1], ptq[:, 32:33],
                                         AF.Exp, scale=-0.8)
                    nc.scalar.activation(b1e[:, q:q + 1], ptq[:, 32:33],
                                         AF.Exp)
                    nc.vector.tensor_scalar(
                        hpa1[:, q, 0:32], ptq[:, 0:32],
                        b1e[:, q:q + 1], None, op0=OP.mult)
                    nc.vector.tensor_copy(hpa1[:, q, 32:33],
                                          b1e[:, q:q + 1])
                nc.vector.memset(hpa1[:, :, 33:34], 0.0)

                # keep PE hot while the s1 AllGather is in flight
                wps2 = tp2.tile([128, 33], F32, tag="ptq")
                for r in range(80):
                    nc.tensor.matmul(wps2[:], wlhsb[:], wsrcb[:, 0:33],
                                     start=(r == 0), stop=(r == 79))

                # gathered s1 -> a1 = e^{0.8 s1} (bf16), bcast per dst chunk
                nc.scalar.activation(a1g[:], s1raw[:], AF.Exp, scale=0.8)

                for p in range(4):
                    for k in range(2):
                        ic = 2 * p + k
                        pa1 = pa1p.tile([128, RPC], F32, tag="pa1")
                        nc.tensor.matmul(pa1[:],
                                         sela[:, ic * 128:(ic + 1) * 128],
                                         a1g[:])
                        nc.vector.tensor_copy(a1t[:, ic, :], pa1[:])
                    pgs = [agg1.tile([33, RPC], F32, name=f"pg1_{p}_{k}",
                                     tag="pg1") for k in range(2)]
                    for jt in range(4):
                        e1 = e1pool.tile([128, 2 * RPC], BF16, tag="e1")
                        nc.vector.tensor_scalar(
                            e1[:].rearrange("p (a b) -> p a b", b=RPC),
                            a1t[:, 2 * p:2 * p + 2, :],
                            g1e[:, jt:jt + 1], None, op0=OP.max)
                        for k in range(2):
                            nc.tensor.matmul(
                                pgs[k][:], hpa1[:, jt, 0:33],
                                e1[:, k * RPC:(k + 1) * RPC],
                                start=(jt == 0), stop=(jt == 3))
                    for k in range(2):
                        rss = rssp.tile([33, RPC], BF16, tag="rss")
                        nc.vector.tensor_copy(rss[0:32, :], pgs[k][0:32, :])
                        nc.vector.tensor_copy(rss[32:33, :],
                                              pgs[k][32:33, :])
                        nc.sync.dma_start(rsin[2 * p + k, :, :], rss[:])

                # keep PE hot across the ReduceScatter
                wps3 = pa1p.tile([128, RPC], F32, tag="pa1")
                for r in range(40):
                    nc.tensor.matmul(wps3[:], wlhsb[:], wsrcb[:],
                                     start=(r == 0), stop=(r == 39))

                nc.gpsimd.collective_compute(
                    "ReduceScatter", OP.add,
                    replica_groups=[list(range(NCORES))],
                    ins=[rsin.opt()], outs=[rsout.opt()])
                nc.sync.dma_start(rso[:], rsout[:])

                # normalize + write out
                nc.scalar.copy(num32[:], rso[0:32, :])
                nc.scalar.copy(rcp1[:], rso[32:33, :])
                nc.vector.reciprocal_approx_fast(rcp1[:], rcp1[:])
                prb1 = agg1.tile([33, RPC], F32, tag="pg1")
                nc.tensor.matmul(prb1[0:32, :], ones32[:], rcp1[:])
                nc.vector.tensor_tensor(outv[:], num32[:], prb1[0:32, :],
                                        op=OP.mult)
                for q in range(4):
                    pt2 = tp2.tile([128, 33], F32, tag="ptq")
                    nc.tensor.matmul(pt2[:, 0:32],
                                     outv[:, q * 128:(q + 1) * 128],
                                     ident[0:32, 0:32], is_transpose=True)
                    ob = otp.tile([128, 32], F32, tag="ob")
                    nc.vector.tensor_copy(ob[:], pt2[:, 0:32])
                    nc.sync.dma_start(out_d[q * 128:(q + 1) * 128, :], ob[:])

    nc.compile()
    return nc


def _fold(inputs):
    """Host-side BN/bias/attention-projection folding (numpy, f64)."""
    f64 = np.float64
    x = np.asarray(inputs["x"], np.float32)
    xT = np.ascontiguousarray(x.T)              # [32, 4096]
    w0 = np.asarray(inputs["w0"], f64)          # [8, 32, 8]
    w1 = np.asarray(inputs["w1"], f64)          # [1, 64, 32]
    a_src0 = np.asarray(inputs["a_src0"], f64)[..., 0]   # [8, 8]
    a_dst0 = np.asarray(inputs["a_dst0"], f64)[..., 0]   # [8, 8]
    a_src1 = np.asarray(inputs["a_src1"], f64)[0, :, 0]  # [32]
    a_dst1 = np.asarray(inputs["a_dst1"], f64)[0, :, 0]  # [32]
    b0 = np.asarray(inputs["b0"], f64)          # [8]
    b1 = np.asarray(inputs["b1"], f64)          # [32]

    al0 = np.asarray(inputs["bn0_gamma"], f64) / np.sqrt(
        np.asarray(inputs["bn0_var"], f64) + BN_EPS)
    sh0 = np.asarray(inputs["bn0_beta"], f64) - \
        np.asarray(inputs["bn0_mean"], f64) * al0
    al1 = np.asarray(inputs["bn1_gamma"], f64) / np.sqrt(
        np.asarray(inputs["bn1_var"], f64) + BN_EPS)
    sh1 = np.asarray(inputs["bn1_beta"], f64) - \
        np.asarray(inputs["bn1_mean"], f64) * al1

    w0flat = (al0[None, :, None] * w0).transpose(1, 0, 2).reshape(32, 64)
    beta0h = np.einsum("i,hio->ho", sh0, w0)     # [8, 8]
    beta0 = (beta0h + b0[None, :]).reshape(64)
    as0 = al0[:, None] * np.einsum("hio,ho->ih", w0, a_src0)   # [32, 8]
    sb0 = np.einsum("ho,ho->h", beta0h, a_src0)
    ad0 = al0[:, None] * np.einsum("hio,ho->ih", w0, a_dst0)
    db0 = np.einsum("ho,ho->h", beta0h, a_dst0)

    w0all = np.zeros((33, 72), f64)
    w0all[0:32, 0:64] = w0flat
    w0all[32, 0:64] = beta0
    w0all[0:32, 64:72] = ad0
    w0all[32, 64:72] = db0
    w0s = np.zeros((33, 8), f64)
    w0s[0:32, :] = as0
    w0s[32, :] = sb0

    # layer 1 folds; input arrives as contp = elu(out0)+1
    w1m = w1[0]                                   # [64, 32]
    w1flat = al1[:, None] * w1m
    beta1 = sh1 @ w1m + b1
    as1 = al1 * (w1m @ a_src1)
    sb1 = (sh1 @ w1m) @ a_src1
    ad1 = al1 * (w1m @ a_dst1)
    db1 = (sh1 @ w1m) @ a_dst1

    w1ext = np.zeros((65, 65), f64)
    w1ext[0:64, 0:32] = w1flat
    w1ext[64, 0:32] = beta1 - w1flat.sum(axis=0)
    w1ext[0:64, 32] = ad1
    w1ext[64, 32] = db1 - ad1.sum()
    w1ext[0:64, 64] = as1
    w1ext[64, 64] = sb1 - as1.sum()

    sela = np.zeros((8, 8, 128), ml_dtypes.bfloat16)
    for h in range(8):
        sela[h, h, :] = 1.0
    s2sel = np.zeros((2, 16), np.float32)
    for p in range(2):
        s2sel[p, p * 8:(p + 1) * 8] = 1.0

    xTo = np.ones((33, N), np.float32)
    xTo[0:32, :] = xT
    return {
        "xT": xTo,
        "w0all": w0all.astype(np.float32),
        "w0s": w0s.astype(np.float32),
        "w1ext": w1ext.astype(np.float32),
        "sela": sela.reshape(8, 8 * 128),
        "s2sel": s2sel,
    }


def kernel(**inputs) -> np.ndarray:
    if "nc" not in _CACHE:
        _CACHE["nc"] = _build()
    nc = _CACHE["nc"]

    shared = _fold(inputs)
    xTo = shared["xT"]
    in_maps = []
    for c in range(NCORES):
        m = dict(shared)
        m["xsT"] = np.ascontiguousarray(xTo[:, c * RPC:(c + 1) * RPC])
        in_maps.append(m)

    res = run_bass_kernel_spmd(nc, in_maps, list(range(NCORES)))
    out = np.concatenate([res.results[c]["out"] for c in range(NCORES)],
                         axis=0)
    return out.astype(np.float32)


# revision 44
# speedup vs baseline: 1.2638x; 1.1309x over previous
"""GAT (2-layer dense-graph attention over 4096 nodes) as a Trainium2
Bass/Tile SPMD kernel across 8 NeuronCores.

Structure:
- Layer 0 DST-sharded (512 destination rows/core, full 4096-source
  h'/d per core). Layer 1 SOURCE-sharded: each core's own 512 layer-0
  output rows are its layer-1 sources; partial numerators/denominators
  for ALL 4096 destinations are summed with one bf16 ReduceScatter.
- Collectives: tiny s1 AllGather (2KB/core) + final ReduceScatter. A
  dummy AllGather at kernel start absorbs the device barrier + CC
  warmup off the critical path.
- x arrives host-transposed (xT [32,4096]) so all DMAs are contiguous
  16KB-per-partition reads - no on-chip transposes of x, no strided
  gather DMA.
- E' = max(e^{0.8 s_i} e^{d_j}, e^{0.2 d_j}) (exact leakyrelu-softmax
  rescale) as ONE DVE tensor_scalar per tile, bf16.
- elu as contp = elu(x)+1 = max(x,0) + min(e^x,1); the -1 folded into
  layer-1 beta rows. All biases/BN folded host-side.
- Reciprocals via DVE reciprocal_approx_fast (~18 bits, one op):
  ScalarE uses only Exp/Copy -> a single ACT table set, zero reloads.
- d0 scores accumulate into a persistent PSUM bank; two batched
  ScalarE exps produce all 256 per-(jt,h) softmax scalars.
- Dummy matmuls keep the PE HAM clock-gate warm across the s1-gather
  and ReduceScatter waits.
"""

import numpy as np
import ml_dtypes

import concourse.bacc as bacc
import concourse.mybir as mybir
import concourse.tile as tile
from concourse import masks
from concourse.bass_utils import run_bass_kernel_spmd

F32 = mybir.dt.float32
BF16 = mybir.dt.bfloat16
AF = mybir.ActivationFunctionType
OP = mybir.AluOpType
N = 4096
NCORES = 8
RPC = N // NCORES          # rows per core = 512
NJT = N // 128             # 32 j-tiles of 128 source rows
BN_EPS = 1e-5

_CACHE = {}


def _build():
    nc = bacc.Bacc("TRN2", target_bir_lowering=False, debug=False,
                   num_devices=NCORES)

    xT_d = nc.dram_tensor("xT", [33, N], F32, kind="ExternalInput")
    xsT_d = nc.dram_tensor("xsT", [33, RPC], F32, kind="ExternalInput")
    w0all_d = nc.dram_tensor("w0all", [33, 72], F32, kind="ExternalInput")
    w0s_d = nc.dram_tensor("w0s", [33, 8], F32, kind="ExternalInput")
    w1ext_d = nc.dram_tensor("w1ext", [65, 65], F32, kind="ExternalInput")
    sela_d = nc.dram_tensor("sela", [8, 8 * 128], BF16, kind="ExternalInput")
    s2sel_d = nc.dram_tensor("s2sel", [2, 16], F32, kind="ExternalInput")
    out_d = nc.dram_tensor("out", [RPC, 32], F32, kind="ExternalOutput")

    with tile.TileContext(nc) as tc:
        with (
            tc.tile_pool(name="const", bufs=1) as const,
            tc.tile_pool(name="per", bufs=1) as per,
            tc.tile_pool(name="psper", bufs=1, space="PSUM") as psper,
            tc.tile_pool(name="dram", bufs=1, space="DRAM") as dram,
        ):
            # ---------- dram intermediates ----------
            dum_i = dram.tile([1, 8], F32, name="dum_i", tag="dum_i")
            dum_o = dram.tile([8, 8], F32, name="dum_o", tag="dum_o")
            s1d = dram.tile([1, RPC], F32, name="s1d", tag="s1d")
            s1g = dram.tile([NCORES, RPC], F32, name="s1g", tag="s1g")
            rsin = dram.tile([NCORES, 33, RPC], BF16, name="rsin", tag="rsin")
            rsout = dram.tile([33, RPC], BF16, name="rsout", tag="rsout")

            # dummy collective first: absorbs device barrier + CC warmup
            nc.gpsimd.collective_compute(
                "AllGather", OP.bypass,
                replica_groups=[list(range(NCORES))],
                ins=[dum_i.opt()], outs=[dum_o.opt()])

            # ---------- consts ----------
            ident = const.tile([128, 128], F32)
            masks.make_identity(nc, ident[:])
            ones512 = const.tile([1, RPC], F32)
            nc.vector.memset(ones512[:], 1.0)
            ones32 = const.tile([1, 32], F32)
            nc.vector.memset(ones32[:], 1.0)
            sela = const.tile([8, 8 * 128], BF16)
            nc.sync.dma_start(sela[:], sela_d[:])
            s2sel = const.tile([2, 16], F32)
            nc.sync.dma_start(s2sel[:], s2sel_d[:])
            w0all = const.tile([33, 72], F32)
            nc.sync.dma_start(w0all[:], w0all_d[:])
            w0s = const.tile([33, 8], F32)
            nc.sync.dma_start(w0s[:], w0s_d[:])
            w1c = [const.tile([16, 65], F32, name=f"w1c{c}", tag=f"w1c{c}")
                   for c in range(4)]
            for c in range(4):
                nc.sync.dma_start(w1c[c][:], w1ext_d[16 * c:16 * c + 16, :])
            w1last = const.tile([1, 65], F32)
            nc.sync.dma_start(w1last[:], w1ext_d[64:65, :])

            # ---------- persistent sbuf ----------
            xT = per.tile([33, N], F32)
            xsT = per.tile([33, RPC], F32)
            # stationary holds h'*e^{d} (cols 0:8) and e^{d} (col 32) so
            # the per-tile DVE op is a SINGLE-scalar max:
            #   E'' = max(e^{0.8 s_i}, e^{-0.8 d_j});  E = e^{d_j} * E''
            hpa0 = per.tile([128, NJT, 8, 34], BF16)
            g0 = per.tile([128, NJT * 8], F32)         # e^{-0.8 d0}
            b0rep = per.tile([128, NJT, 8, 8], F32)    # e^{d0} rep x8
            hp0f = per.tile([128, NJT, 64], F32)       # h'0 staging
            atile = per.tile([128, 8, RPC], BF16)      # e^{0.8 s0} bcast
            nums = per.tile([16, 4, RPC], F32)
            dens = per.tile([2, 4, RPC], F32)
            contp = per.tile([16, 4, RPC], F32)        # elu(out0)+1 chunks
            hp1s = per.tile([33, RPC], F32)
            hpa1 = per.tile([128, 4, 34], BF16)
            g1e = per.tile([128, 4], F32)              # e^{-0.8 d1}
            b1e = per.tile([128, 4], F32)              # e^{d1}
            s1s = per.tile([1, RPC], F32)
            s1raw = per.tile([8, RPC], F32)
            a1g = per.tile([8, RPC], BF16)
            a1t = per.tile([128, 8, RPC], BF16)
            num32 = per.tile([32, RPC], F32)
            outv = per.tile([32, RPC], F32)
            rso = per.tile([33, RPC], BF16)
            rcp1 = per.tile([1, RPC], F32)

            ps1 = psper.tile([65, RPC], F32)
            hT72 = per.tile([72, N], F32)              # h'0/d0 row-major
            d0raw = per.tile([128, NJT * 8], F32)      # d0 scores, j-major

            # ---------------- Phase A: warmup + prep ----------------
            with (
                tc.tile_pool(name="ld", bufs=2) as ld,
                tc.tile_pool(name="mm72", bufs=2, space="PSUM") as mm72,
                tc.tile_pool(name="trp", bufs=2, space="PSUM") as trp,
                tc.tile_pool(name="ps0p", bufs=1, space="PSUM") as ps0p,
                tc.tile_pool(name="pab", bufs=2, space="PSUM") as pab,
            ):
                wsrc = ld.tile([128, 512], BF16, tag="wsrc")
                nc.vector.memset(wsrc[:], 0.5)
                wlhs = ld.tile([128, 128], BF16, tag="wlhs")
                nc.vector.memset(wlhs[:], 0.25)
                wps = pab.tile([128, RPC], F32, tag="pa")
                for r in range(14):
                    nc.tensor.matmul(wps[:], wlhs[:], wsrc[:],
                                     start=(r == 0), stop=(r == 13))

                # host-transposed inputs (ones row baked in on host):
                # contiguous big-granule DMAs, zero on-chip fixup
                nc.sync.dma_start(xT[:], xT_d[:])
                nc.sync.dma_start(xsT[:], xsT_d[:])

                # s0 for own 512 dst rows; atile = e^{0.8 s0} bcast
                ps0 = ps0p.tile([8, RPC], F32, tag="ps0")
                nc.tensor.matmul(ps0[:], w0s[:], xsT[:])
                a0row = ld.tile([8, RPC], BF16, tag="a0row")
                nc.scalar.activation(a0row[:], ps0[:], AF.Exp, scale=0.8)
                for h in range(8):
                    pa = pab.tile([128, RPC], F32, tag="pa")
                    nc.tensor.matmul(pa[:], sela[:, h * 128:(h + 1) * 128],
                                     a0row[:])
                    nc.scalar.copy(atile[:, h, :], pa[:])

                # h'0/d0 for all 4096 sources: ONE 72-col stationary
                # (w0all), xT streamed through in 8 chunks -> row-major
                # [72, 4096]; PE transposes bring it back j-on-partitions
                nc.vector.memset(hpa0[:], 0.0)
                nc.vector.memset(hpa0[:, :, :, 32:33], 1.0)
                for cc in range(8):
                    p72 = mm72.tile([72, 512], F32, tag="p72")
                    nc.tensor.matmul(p72[:], w0all[:],
                                     xT[:, cc * 512:(cc + 1) * 512])
                    nc.scalar.copy(hT72[:, cc * 512:(cc + 1) * 512], p72[:])
                for jt in range(NJT):
                    ptr = trp.tile([128, 72], F32, tag="ptr")
                    nc.tensor.matmul(ptr[:],
                                     hT72[:, jt * 128:(jt + 1) * 128],
                                     ident[0:72, 0:72], is_transpose=True)
                    nc.vector.tensor_copy(hp0f[:, jt, :], ptr[:, 0:64])
                    nc.vector.tensor_copy(d0raw[:, jt * 8:(jt + 1) * 8],
                                          ptr[:, 64:72])
                    if jt % 8 == 7:
                        g = jt // 8
                        gs = slice(g * 64, (g + 1) * 64)
                        gb = slice(g * 8, (g + 1) * 8)
                        draw = d0raw[:, gs].rearrange(
                            "p (a b) -> p a b", b=8)
                        nc.scalar.activation(g0[:, gs], d0raw[:, gs],
                                             AF.Exp, scale=-0.8)
                        for o in range(4):
                            nc.scalar.activation(
                                b0rep[:, gb, :, o], draw, AF.Exp)
                        for o in range(4, 8):
                            nc.vector.tensor_copy(
                                b0rep[:, gb, :, o], b0rep[:, gb, :, 0])
                        nc.scalar.activation(
                            hpa0[:, gb, :, 32], draw, AF.Exp)
                        nc.vector.tensor_tensor(
                            hpa0[:, gb, :, 0:8],
                            hp0f[:, gb, :].rearrange(
                                "p a (h o) -> p a h o", h=8),
                            b0rep[:, gb, :, :], op=OP.mult)

            # ---------------- Phase B: layer-0 attention ----------------
            with (
                tc.tile_pool(name="epool", bufs=8) as epool,
                tc.tile_pool(name="agg", bufs=3, space="PSUM") as agg,
                tc.tile_pool(name="rb", bufs=1, space="PSUM") as rb,
                tc.tile_pool(name="tmp", bufs=2) as tmp,
            ):
                for h in range(8):
                    ch, hh = h // 2, h % 2
                    pg = agg.tile([33, RPC], F32)
                    for jt in range(NJT):
                        e = epool.tile([128, RPC], BF16, tag="e")
                        nc.vector.tensor_scalar(
                            e[:], atile[:, h, :],
                            g0[:, jt * 8 + h:jt * 8 + h + 1], None,
                            op0=OP.max)
                        nc.tensor.matmul(pg[:], hpa0[:, jt, h, 0:33], e[:],
                                         start=(jt == 0), stop=(jt == NJT - 1))
                    stg = tmp.tile([8, RPC], F32, tag="stg")
                    nc.scalar.copy(stg[:], pg[0:8, :])
                    std = tmp.tile([1, RPC], F32, tag="std")
                    nc.scalar.copy(std[:], pg[32:33, :])
                    nc.sync.dma_start(nums[hh * 8:(hh + 1) * 8, ch, :], stg[:])
                    nc.sync.dma_start(dens[hh:hh + 1, ch, :], std[:])

                    if hh == 1:
                        # chunk complete: normalize + (elu+1) -> contp
                        rcp = tmp.tile([2, RPC], F32, tag="rcp")
                        nc.vector.reciprocal_approx_fast(rcp[:], dens[:, ch, :])
                        prbc = rb.tile([16, RPC], F32)
                        nc.tensor.matmul(prbc[:], s2sel[:], rcp[:])
                        nrm = tmp.tile([16, RPC], F32, tag="nrm")
                        nc.vector.tensor_tensor(nrm[:], nums[:, ch, :],
                                                prbc[:], op=OP.mult)
                        texp = tmp.tile([16, RPC], F32, tag="texp")
                        nc.scalar.activation(texp[:], nrm[:], AF.Exp)
                        t1 = tmp.tile([16, RPC], F32, tag="t1")
                        nc.vector.tensor_scalar_min(t1[:], texp[:], 1.0)
                        nc.vector.scalar_tensor_tensor(
                            contp[:, ch, :], nrm[:], 0.0, t1[:],
                            op0=OP.max, op1=OP.add)
                        # accumulate s1 / h'1 / d1 into ps1
                        nc.tensor.matmul(ps1[:], w1c[ch][:], contp[:, ch, :],
                                         start=(ch == 0), stop=(ch == 3))
                        if ch == 0:
                            nc.tensor.matmul(ps1[:], w1last[:], ones512[:],
                                             start=False, stop=False)

            # ---------------- Phase C: inter-layer + layer 1 ----------------
            with (
                tc.tile_pool(name="ld2", bufs=2) as ld2,
                tc.tile_pool(name="tp2", bufs=2, space="PSUM") as tp2,
                tc.tile_pool(name="pa1p", bufs=2, space="PSUM") as pa1p,
                tc.tile_pool(name="agg1", bufs=2, space="PSUM") as agg1,
                tc.tile_pool(name="e1pool", bufs=4) as e1pool,
                tc.tile_pool(name="rssp", bufs=2) as rssp,
                tc.tile_pool(name="otp", bufs=2) as otp,
            ):
                # bridge the terminal-chunk serial chain so HAM stays hot
                wsrcb = ld2.tile([128, 512], BF16, tag="wsrcb")
                nc.vector.memset(wsrcb[:], 0.5)
                wlhsb = ld2.tile([128, 128], BF16, tag="wlhsb")
                nc.vector.memset(wlhsb[:], 0.25)
                wpsb = pa1p.tile([128, RPC], F32, tag="pa1")
                for r in range(16):
                    nc.tensor.matmul(wpsb[:], wlhsb[:], wsrcb[:],
                                     start=(r == 0), stop=(r == 15))

                # s1 out the door first: evac -> DRAM -> AllGather
                nc.scalar.copy(s1s[:], ps1[64:65, :])
                nc.sync.dma_start(s1d[:], s1s[:])
                nc.gpsimd.collective_compute(
                    "AllGather", OP.bypass,
                    replica_groups=[list(range(NCORES))],
                    ins=[s1d.opt()], outs=[s1g.opt()])
                nc.sync.dma_start(s1raw[:], s1g[:])

                # local h'1 -> transposed bf16 stationary + d1 exps
                nc.scalar.copy(hp1s[0:32, :], ps1[0:32, :])
                nc.scalar.copy(hp1s[32:33, :], ps1[32:33, :])
                for q in range(4):
                    ptq = tp2.tile([128, 33], F32, tag="ptq")
                    nc.tensor.matmul(ptq[:],
                                     hp1s[:, q * 128:(q + 1) * 128],
                                     ident[0:33, 0:33], is_transpose=True)
                    nc.scalar.activation(g1e[:, q:q + 1], ptq[:, 32:33],
                                         AF.Exp, scale=-0.8)
                    nc.scalar.activation(b1e[:, q:q + 1], ptq[:, 32:33],
                                         AF.Exp)
                    nc.vector.tensor_scalar(
                        hpa1[:, q, 0:32], ptq[:, 0:32],
                        b1e[:, q:q + 1], None, op0=OP.mult)
                    nc.vector.tensor_copy(hpa1[:, q, 32:33],
                                          b1e[:, q:q + 1])
                nc.vector.memset(hpa1[:, :, 33:34], 0.0)

                # keep PE hot while the s1 AllGather is in flight
                wps2 = tp2.tile([128, 33], F32, tag="ptq")
                for r in range(80):
                    nc.tensor.matmul(wps2[:], wlhsb[:], wsrcb[:, 0:33],
                                     start=(r == 0), stop=(r == 79))

                # gathered s1 -> a1 = e^{0.8 s1} (bf16), bcast per dst chunk
                nc.scalar.activation(a1g[:], s1raw[:], AF.Exp, scale=0.8)

                for p in range(4):
                    for k in range(2):
                        ic = 2 * p + k
                        pa1 = pa1p.tile([128, RPC], F32, tag="pa1")
                        nc.tensor.matmul(pa1[:],
                                         sela[:, ic * 128:(ic + 1) * 128],
                                         a1g[:])
                        nc.scalar.copy(a1t[:, ic, :], pa1[:])
                    pgs = [agg1.tile([33, RPC], F32, name=f"pg1_{p}_{k}",
                                     tag="pg1") for k in range(2)]
                    for jt in range(4):
                        e1 = e1pool.tile([128, 2 * RPC], BF16, tag="e1")
                        nc.vector.tensor_scalar(
                            e1[:].rearrange("p (a b) -> p a b", b=RPC),
                            a1t[:, 2 * p:2 * p + 2, :],
                            g1e[:, jt:jt + 1], None, op0=OP.max)
                        for k in range(2):
                            nc.tensor.matmul(
                                pgs[k][:], hpa1[:, jt, 0:33],
                                e1[:, k * RPC:(k + 1) * RPC],
                                start=(jt == 0), stop=(jt == 3))
                    for k in range(2):
                        rss = rssp.tile([33, RPC], BF16, tag="rss")
                        nc.vector.tensor_copy(rss[0:32, :], pgs[k][0:32, :])
                        nc.scalar.copy(rss[32:33, :], pgs[k][32:33, :])
                        nc.sync.dma_start(rsin[2 * p + k, :, :], rss[:])

                # keep PE hot across the ReduceScatter
                wps3 = pa1p.tile([128, RPC], F32, tag="pa1")
                for r in range(40):
                    nc.tensor.matmul(wps3[:], wlhsb[:], wsrcb[:],
                                     start=(r == 0), stop=(r == 39))

                nc.gpsimd.collective_compute(
                    "ReduceScatter", OP.add,
                    replica_groups=[list(range(NCORES))],
                    ins=[rsin.opt()], outs=[rsout.opt()])
                nc.sync.dma_start(rso[:], rsout[:])

                # normalize + write out
                nc.scalar.copy(num32[:], rso[0:32, :])
                nc.scalar.copy(rcp1[:], rso[32:33, :])
                nc.vector.reciprocal_approx_fast(rcp1[:], rcp1[:])
                prb1 = agg1.tile([33, RPC], F32, tag="pg1")
                nc.tensor.matmul(prb1[0:32, :], ones32[:], rcp1[:])
                nc.vector.tensor_tensor(outv[:], num32[:], prb1[0:32, :],
                                        op=OP.mult)
                for q in range(4):
                    pt2 = tp2.tile([128, 33], F32, tag="ptq")
                    nc.tensor.matmul(pt2[:, 0:32],
                                     outv[:, q * 128:(q + 1) * 128],
                                     ident[0:32, 0:32], is_transpose=True)
                    ob = otp.tile([128, 32], F32, tag="ob")
                    nc.vector.tensor_copy(ob[:], pt2[:, 0:32])
                    nc.sync.dma_start(out_d[q * 128:(q + 1) * 128, :], ob[:])

    nc.compile()
    return nc


def _fold(inputs):
    """Host-side BN/bias/attention-projection folding (numpy, f64)."""
    f64 = np.float64
    x = np.asarray(inputs["x"], np.float32)
    xT = np.ascontiguousarray(x.T)              # [32, 4096]
    w0 = np.asarray(inputs["w0"], f64)          # [8, 32, 8]
    w1 = np.asarray(inputs["w1"], f64)          # [1, 64, 32]
    a_src0 = np.asarray(inputs["a_src0"], f64)[..., 0]   # [8, 8]
    a_dst0 = np.asarray(inputs["a_dst0"], f64)[..., 0]   # [8, 8]
    a_src1 = np.asarray(inputs["a_src1"], f64)[0, :, 0]  # [32]
    a_dst1 = np.asarray(inputs["a_dst1"], f64)[0, :, 0]  # [32]
    b0 = np.asarray(inputs["b0"], f64)          # [8]
    b1 = np.asarray(inputs["b1"], f64)          # [32]

    al0 = np.asarray(inputs["bn0_gamma"], f64) / np.sqrt(
        np.asarray(inputs["bn0_var"], f64) + BN_EPS)
    sh0 = np.asarray(inputs["bn0_beta"], f64) - \
        np.asarray(inputs["bn0_mean"], f64) * al0
    al1 = np.asarray(inputs["bn1_gamma"], f64) / np.sqrt(
        np.asarray(inputs["bn1_var"], f64) + BN_EPS)
    sh1 = np.asarray(inputs["bn1_beta"], f64) - \
        np.asarray(inputs["bn1_mean"], f64) * al1

    w0flat = (al0[None, :, None] * w0).transpose(1, 0, 2).reshape(32, 64)
    beta0h = np.einsum("i,hio->ho", sh0, w0)     # [8, 8]
    beta0 = (beta0h + b0[None, :]).reshape(64)
    as0 = al0[:, None] * np.einsum("hio,ho->ih", w0, a_src0)   # [32, 8]
    sb0 = np.einsum("ho,ho->h", beta0h, a_src0)
    ad0 = al0[:, None] * np.einsum("hio,ho->ih", w0, a_dst0)
    db0 = np.einsum("ho,ho->h", beta0h, a_dst0)

    w0all = np.zeros((33, 72), f64)
    w0all[0:32, 0:64] = w0flat
    w0all[32, 0:64] = beta0
    w0all[0:32, 64:72] = ad0
    w0all[32, 64:72] = db0
    w0s = np.zeros((33, 8), f64)
    w0s[0:32, :] = as0
    w0s[32, :] = sb0

    # layer 1 folds; input arrives as contp = elu(out0)+1
    w1m = w1[0]                                   # [64, 32]
    w1flat = al1[:, None] * w1m
    beta1 = sh1 @ w1m + b1
    as1 = al1 * (w1m @ a_src1)
    sb1 = (sh1 @ w1m) @ a_src1
    ad1 = al1 * (w1m @ a_dst1)
    db1 = (sh1 @ w1m) @ a_dst1

    w1ext = np.zeros((65, 65), f64)
    w1ext[0:64, 0:32] = w1flat
    w1ext[64, 0:32] = beta1 - w1flat.sum(axis=0)
    w1ext[0:64, 32] = ad1
    w1ext[64, 32] = db1 - ad1.sum()
    w1ext[0:64, 64] = as1
    w1ext[64, 64] = sb1 - as1.sum()

    sela = np.zeros((8, 8, 128), ml_dtypes.bfloat16)
    for h in range(8):
        sela[h, h, :] = 1.0
    s2sel = np.zeros((2, 16), np.float32)
    for p in range(2):
        s2sel[p, p * 8:(p + 1) * 8] = 1.0

    xTo = np.ones((33, N), np.float32)
    xTo[0:32, :] = xT
    return {
        "xT": xTo,
        "w0all": w0all.astype(np.float32),
        "w0s": w0s.astype(np.float32),
        "w1ext": w1ext.astype(np.float32),
        "sela": sela.reshape(8, 8 * 128),
        "s2sel": s2sel,
    }


def kernel(**inputs) -> np.ndarray:
    if "nc" not in _CACHE:
        _CACHE["nc"] = _build()
    nc = _CACHE["nc"]

    shared = _fold(inputs)
    xTo = shared["xT"]
    in_maps = []
    for c in range(NCORES):
        m = dict(shared)
        m["xsT"] = np.ascontiguousarray(xTo[:, c * RPC:(c + 1) * RPC])
        in_maps.append(m)

    res = run_bass_kernel_spmd(nc, in_maps, list(range(NCORES)))
    out = np.concatenate([res.results[c]["out"] for c in range(NCORES)],
                         axis=0)
    return out.astype(np.float32)
